# revision 4
# baseline (speedup 1.0000x reference)
"""Trainium2 kernel: y = relu((x - pb) @ W + b) with per-row top-K threshold masking.

Strategy (per spec sharding hint): data-parallel over rows across 8 cores.
Each core computes its row shard with a 3-pass bf16 matmul decomposition
(x_hi@W_hi + x_hi@W_lo + x_lo@W_hi, f32 PSUM accumulation, ~1e-5 accurate),
then finds each row's K-th largest activation by a fused count binary search
on DVE (tensor_scalar is_ge + accumulate), which converges to 1 ulp and
reproduces jax.lax.top_k threshold masking exactly (ties included).
"""
import sys
sys.path.insert(0, "/opt/trn_rl_repo")

import numpy as np
import concourse.bass as bass
import concourse.bacc as bacc
import concourse.mybir as mybir
from concourse.tile import TileContext
from concourse.masks import make_identity

F32 = mybir.dt.float32
BF16 = mybir.dt.bfloat16

# full problem dims (hardcoded; kernel.py must be self-contained)
B_FULL, D_IN, N_FEAT, K_TOP = 16384, 4096, 4096, 128
N_CORES = 8


def build_nc(B_core, D, F, K, n_iters=27, super_size=2, fb=512, debug_acts=False):
    assert B_core % 128 == 0 and D % 128 == 0 and F % fb == 0
    nc = bacc.Bacc("TRN2", target_bir_lowering=False, debug=True)
    x = nc.dram_tensor("x", [B_core, D], F32, kind="ExternalInput")
    w = nc.dram_tensor("w", [D, F], F32, kind="ExternalInput")
    out = nc.dram_tensor("out", [B_core, F], F32, kind="ExternalOutput")
    acts_dbg = None
    if debug_acts:
        acts_dbg = nc.dram_tensor("acts_dbg", [B_core, F], F32, kind="ExternalOutput")

    n_r = B_core // 128   # row blocks
    n_d = D // 128        # contraction blocks
    n_fb = F // fb        # feature blocks
    supers = [list(range(i, min(i + super_size, n_r)))
              for i in range(0, n_r, super_size)]

    with TileContext(nc) as tc:
        from contextlib import ExitStack
        ctx = ExitStack()
        cpool = ctx.enter_context(tc.tile_pool(name="const", bufs=1))
        dpool = ctx.enter_context(tc.tile_pool(name="wdram", bufs=1, space="DRAM"))
        xr_pool = ctx.enter_context(tc.tile_pool(name="xr", bufs=2))
        xsp_pool = ctx.enter_context(tc.tile_pool(name="xsp", bufs=2 * super_size))
        xt_pool = ctx.enter_context(tc.tile_pool(name="xt", bufs=2 * n_d))
        w_pool = ctx.enter_context(tc.tile_pool(name="wp", bufs=6))
        wsrc_pool = ctx.enter_context(tc.tile_pool(name="wsrc", bufs=2))
        acts_pool = ctx.enter_context(tc.tile_pool(name="acts", bufs=super_size + 1))
        scr_pool = ctx.enter_context(tc.tile_pool(name="scr", bufs=2))
        sm_pool = ctx.enter_context(tc.tile_pool(name="sm", bufs=4 * 6))
        mm_pool = ctx.enter_context(tc.tile_pool(name="mm", bufs=super_size + 1, space="PSUM"))
        tp_pool = ctx.enter_context(tc.tile_pool(name="tp", bufs=2, space="PSUM"))

        ident = cpool.tile([128, 128], BF16)
        make_identity(nc, ident[:])

        wh_d = dpool.tile([D, F], BF16)
        wl_d = dpool.tile([D, F], BF16)

        for si, sup in enumerate(supers):
            ns = len(sup)
            # ---- split x rows into bf16 hi/lo, then transpose via PE ----
            xh_rows, xl_rows = [], []
            for r in sup:
                xr = xr_pool.tile([128, D], F32)
                nc.sync.dma_start(out=xr[:], in_=x[r * 128:(r + 1) * 128, :])
                xh = xsp_pool.tile([128, D], BF16, tag="xsp")
                nc.vector.tensor_copy(xh[:], xr[:])
                xl = xsp_pool.tile([128, D], BF16, tag="xsp")
                nc.vector.tensor_tensor(out=xl[:], in0=xr[:], in1=xh[:],
                                        op=mybir.AluOpType.subtract)
                xh_rows.append(xh)
                xl_rows.append(xl)
            xhT, xlT = [], []
            for db in range(n_d):
                dsl = slice(db * 128, (db + 1) * 128)
                ph = tp_pool.tile([128, ns * 128], BF16, tag="tp")
                for i in range(ns):
                    nc.tensor.transpose(ph[:, i * 128:(i + 1) * 128],
                                        xh_rows[i][:, dsl], ident[:])
                th = xt_pool.tile([128, ns * 128], BF16, tag="xt")
                nc.scalar.copy(th[:], ph[:])
                xhT.append(th)
                pl = tp_pool.tile([128, ns * 128], BF16, tag="tp")
                for i in range(ns):
                    nc.tensor.transpose(pl[:, i * 128:(i + 1) * 128],
                                        xl_rows[i][:, dsl], ident[:])
                tl = xt_pool.tile([128, ns * 128], BF16, tag="xt")
                nc.scalar.copy(tl[:], pl[:])
                xlT.append(tl)

            # ---- 3-pass matmul over feature blocks ----
            acts = [acts_pool.tile([128, F], F32, tag="acts", name=f"acts{_i}") for _i in range(ns)]
            for f in range(n_fb):
                fsl = slice(f * fb, (f + 1) * fb)
                pms = [mm_pool.tile([128, fb], F32, tag="mm", name=f"pm{_i}") for _i in range(ns)]
                for db in range(n_d):
                    dsl = slice(db * 128, (db + 1) * 128)
                    if si == 0:
                        # split W on the fly; cache bf16 pieces in DRAM
                        wsrc = wsrc_pool.tile([128, fb], F32)
                        nc.sync.dma_start(out=wsrc[:], in_=w[dsl, fsl])
                        wh_sb = w_pool.tile([128, fb], BF16, tag="wp")
                        nc.vector.tensor_copy(wh_sb[:], wsrc[:])
                        wl_sb = w_pool.tile([128, fb], BF16, tag="wp")
                        nc.vector.tensor_tensor(out=wl_sb[:], in0=wsrc[:], in1=wh_sb[:],
                                                op=mybir.AluOpType.subtract)
                        nc.sync.dma_start(out=wh_d[dsl, fsl], in_=wh_sb[:])
                        nc.sync.dma_start(out=wl_d[dsl, fsl], in_=wl_sb[:])
                    else:
                        wh_sb = w_pool.tile([128, fb], BF16, tag="wp")
                        nc.sync.dma_start(out=wh_sb[:], in_=wh_d[dsl, fsl])
                        wl_sb = w_pool.tile([128, fb], BF16, tag="wp")
                        nc.sync.dma_start(out=wl_sb[:], in_=wl_d[dsl, fsl])
                    last = db == n_d - 1
                    for i in range(ns):
                        isl = slice(i * 128, (i + 1) * 128)
                        nc.tensor.matmul(pms[i][:], xhT[db][:, isl], wh_sb[:],
                                         start=(db == 0), stop=False)
                        nc.tensor.matmul(pms[i][:], xhT[db][:, isl], wl_sb[:],
                                         start=False, stop=False)
                        nc.tensor.matmul(pms[i][:], xlT[db][:, isl], wh_sb[:],
                                         start=False, stop=last)
                for i in range(ns):
                    nc.scalar.activation(acts[i][:, fsl], pms[i][:],
                                         mybir.ActivationFunctionType.Relu)

            if debug_acts:
                for i, r in enumerate(sup):
                    nc.sync.dma_start(out=acts_dbg[r * 128:(r + 1) * 128, :],
                                      in_=acts[i][:])

            # ---- per-row K-th largest via count binary search ----
            # state: lo (threshold lower bound), wdt (interval width); hi = lo + wdt
            # invariant: count(acts >= lo) >= K, count(acts >= lo + wdt) < K
            lo = sm_pool.tile([128, ns], F32, tag="sm")
            nc.vector.memset(lo[:], 0.0)
            wdt = sm_pool.tile([128, ns], F32, tag="sm")
            for i in range(ns):
                nc.vector.reduce_max(out=wdt[:, i:i + 1], in_=acts[i][:],
                                     axis=mybir.AxisListType.X)
            nc.vector.tensor_scalar_add(wdt[:], wdt[:], 1.0)
            mid = sm_pool.tile([128, ns], F32, tag="sm")
            nc.vector.tensor_scalar_mul(mid[:], wdt[:], 0.5)
            cnt = sm_pool.tile([128, ns], F32, tag="sm")
            tgw = sm_pool.tile([128, ns], F32, tag="sm")
            for it in range(n_iters):
                for i in range(ns):
                    scr = scr_pool.tile([128, F], BF16, tag="scr")
                    nc.vector.tensor_scalar(scr[:], acts[i][:], mid[:, i:i + 1], None,
                                            op0=mybir.AluOpType.is_ge,
                                            op1=mybir.AluOpType.add,
                                            accum_out=cnt[:, i:i + 1])
                # wdt *= 0.5 ; lo += (cnt >= K) * wdt ; mid = 0.5*wdt + lo
                nc.vector.tensor_scalar_mul(wdt[:], wdt[:], 0.5)
                nc.vector.scalar_tensor_tensor(out=tgw[:], in0=cnt[:], scalar=float(K),
                                               in1=wdt[:], op0=mybir.AluOpType.is_ge,
                                               op1=mybir.AluOpType.mult)
                nc.vector.tensor_tensor(out=lo[:], in0=lo[:], in1=tgw[:],
                                        op=mybir.AluOpType.add)
                if it != n_iters - 1:
                    nc.vector.scalar_tensor_tensor(out=mid[:], in0=wdt[:], scalar=0.5,
                                                   in1=lo[:], op0=mybir.AluOpType.mult,
                                                   op1=mybir.AluOpType.add)
            # ---- apply mask: out = acts * (acts >= lo) ----
            for i, r in enumerate(sup):
                nc.vector.scalar_tensor_tensor(out=acts[i][:], in0=acts[i][:],
                                               scalar=lo[:, i:i + 1], in1=acts[i][:],
                                               op0=mybir.AluOpType.is_ge,
                                               op1=mybir.AluOpType.mult)
                nc.sync.dma_start(out=out[r * 128:(r + 1) * 128, :], in_=acts[i][:])
        ctx.close()

    nc.finalize()
    return nc


_NC_CACHE = {}


def _get_nc(key):
    if key not in _NC_CACHE:
        _NC_CACHE[key] = build_nc(*key)
    return _NC_CACHE[key]


def kernel(x, preencoder_bias, W_enc, b_enc):
    from concourse.bass_utils import run_bass_kernel_spmd
    x = np.asarray(x, dtype=np.float32)
    W = np.asarray(W_enc, dtype=np.float32)
    pb = np.asarray(preencoder_bias, dtype=np.float32)
    b = np.asarray(b_enc, dtype=np.float32)

    B, D = x.shape
    F = W.shape[1]
    assert (B, D, F) == (B_FULL, D_IN, N_FEAT)
    # fold biases: (x - pb) @ W + b == x @ W + (b - pb @ W)
    c = (b - pb @ W).astype(np.float32)
    if np.any(c != 0.0):
        # exact: augment the contraction with one extra 128-block where
        # x_aug[:, D] = 1 and W_aug[D, :] = c (rest zeros)
        x_aug = np.zeros((B, D + 128), dtype=np.float32)
        x_aug[:, :D] = x
        x_aug[:, D] = 1.0
        W_aug = np.zeros((D + 128, F), dtype=np.float32)
        W_aug[:D] = W
        W_aug[D] = c
        x, W, D = x_aug, W_aug, D + 128

    B_core = B // N_CORES
    nc = _get_nc((B_core, D, F, K_TOP))
    in_maps = [{"x": np.ascontiguousarray(x[i * B_core:(i + 1) * B_core]), "w": W}
               for i in range(N_CORES)]
    res = run_bass_kernel_spmd(nc, in_maps, core_ids=list(range(N_CORES)))
    return np.concatenate([res.results[i]["out"] for i in range(N_CORES)], axis=0)


# revision 8
# speedup vs baseline: 1.5589x; 1.5589x over previous
"""Trainium2 kernel: y = relu((x - pb) @ W + b) with per-row top-K threshold masking.

Strategy (per spec sharding hint): data-parallel over rows across 8 cores.
Each core computes its row shard with a 3-pass bf16 matmul decomposition
(x_hi@W_hi + x_hi@W_lo + x_lo@W_hi, f32 PSUM accumulation, ~1e-5 accurate),
then finds each row's K-th largest activation by a fused count binary search
on DVE (tensor_scalar is_ge + accumulate), which converges to 1 ulp and
reproduces jax.lax.top_k threshold masking exactly (ties included).
"""
import sys
sys.path.insert(0, "/opt/trn_rl_repo")

import numpy as np
import concourse.bass as bass
import concourse.bacc as bacc
import concourse.mybir as mybir
from concourse.tile import TileContext
from concourse.masks import make_identity

F32 = mybir.dt.float32
BF16 = mybir.dt.bfloat16

# full problem dims (hardcoded; kernel.py must be self-contained)
B_FULL, D_IN, N_FEAT, K_TOP = 16384, 4096, 4096, 128
N_CORES = 8


def build_nc(B_core, D, F, K, n_iters=25, super_size=2, fb=512, debug_acts=False):
    assert B_core % 128 == 0 and D % 128 == 0 and F % fb == 0
    nc = bacc.Bacc("TRN2", target_bir_lowering=False, debug=True)
    x = nc.dram_tensor("x", [B_core, D], F32, kind="ExternalInput")
    w = nc.dram_tensor("w", [D, F], F32, kind="ExternalInput")
    out = nc.dram_tensor("out", [B_core, F], F32, kind="ExternalOutput")
    acts_dbg = None
    if debug_acts:
        acts_dbg = nc.dram_tensor("acts_dbg", [B_core, F], F32, kind="ExternalOutput")

    n_r = B_core // 128   # row blocks
    n_d = D // 128        # contraction blocks
    n_fb = F // fb        # feature blocks
    supers = [list(range(i, min(i + super_size, n_r)))
              for i in range(0, n_r, super_size)]

    with TileContext(nc) as tc:
        from contextlib import ExitStack
        ctx = ExitStack()
        cpool = ctx.enter_context(tc.tile_pool(name="const", bufs=1))
        dpool = ctx.enter_context(tc.tile_pool(name="wdram", bufs=1, space="DRAM"))
        xr_pool = ctx.enter_context(tc.tile_pool(name="xr", bufs=2))
        xsp_pool = ctx.enter_context(tc.tile_pool(name="xsp", bufs=2 * super_size))
        xt_pool = ctx.enter_context(tc.tile_pool(name="xt", bufs=4 * n_d))
        w_pool = ctx.enter_context(tc.tile_pool(name="wp", bufs=6))
        wsrc_pool = ctx.enter_context(tc.tile_pool(name="wsrc", bufs=2))
        acts_pool = ctx.enter_context(tc.tile_pool(name="acts", bufs=super_size + 1))
        scr_pool = ctx.enter_context(tc.tile_pool(name="scr", bufs=1))
        sm_pool = ctx.enter_context(tc.tile_pool(name="sm", bufs=4 * 6))
        mm_pool = ctx.enter_context(tc.tile_pool(name="mm", bufs=super_size + 1, space="PSUM"))
        tp_pool = ctx.enter_context(tc.tile_pool(name="tp", bufs=2, space="PSUM"))

        ident = cpool.tile([128, 128], BF16)
        make_identity(nc, ident[:])

        wh_d = dpool.tile([D, F], BF16)
        wl_d = dpool.tile([D, F], BF16)

        for si, sup in enumerate(supers):
            ns = len(sup)
            # ---- split x rows into bf16 hi/lo, then transpose via PE ----
            xh_rows, xl_rows = [], []
            for r in sup:
                xr = xr_pool.tile([128, D], F32)
                nc.sync.dma_start(out=xr[:], in_=x[r * 128:(r + 1) * 128, :])
                xh = xsp_pool.tile([128, D], BF16, tag="xsp")
                nc.vector.tensor_copy(xh[:], xr[:])
                xl = xsp_pool.tile([128, D], BF16, tag="xsp")
                nc.vector.tensor_tensor(out=xl[:], in0=xr[:], in1=xh[:],
                                        op=mybir.AluOpType.subtract)
                xh_rows.append(xh)
                xl_rows.append(xl)
            xhT, xlT = [], []
            for db in range(n_d):
                dsl = slice(db * 128, (db + 1) * 128)
                ph = tp_pool.tile([128, ns * 128], BF16, tag="tp")
                for i in range(ns):
                    nc.tensor.transpose(ph[:, i * 128:(i + 1) * 128],
                                        xh_rows[i][:, dsl], ident[:])
                th = xt_pool.tile([128, ns * 128], BF16, tag="xt")
                nc.scalar.copy(th[:], ph[:])
                xhT.append(th)
                pl = tp_pool.tile([128, ns * 128], BF16, tag="tp")
                for i in range(ns):
                    nc.tensor.transpose(pl[:, i * 128:(i + 1) * 128],
                                        xl_rows[i][:, dsl], ident[:])
                tl = xt_pool.tile([128, ns * 128], BF16, tag="xt")
                nc.scalar.copy(tl[:], pl[:])
                xlT.append(tl)

            # ---- 3-pass matmul over feature blocks ----
            acts = [acts_pool.tile([128, F], F32, tag="acts", name=f"acts{_i}") for _i in range(ns)]
            for f in range(n_fb):
                fsl = slice(f * fb, (f + 1) * fb)
                pms = [mm_pool.tile([128, fb], F32, tag="mm", name=f"pm{_i}") for _i in range(ns)]
                for db in range(n_d):
                    dsl = slice(db * 128, (db + 1) * 128)
                    if si == 0:
                        # split W on the fly; cache bf16 pieces in DRAM
                        wsrc = wsrc_pool.tile([128, fb], F32)
                        nc.sync.dma_start(out=wsrc[:], in_=w[dsl, fsl])
                        wh_sb = w_pool.tile([128, fb], BF16, tag="wp")
                        nc.vector.tensor_copy(wh_sb[:], wsrc[:])
                        wl_sb = w_pool.tile([128, fb], BF16, tag="wp")
                        nc.vector.tensor_tensor(out=wl_sb[:], in0=wsrc[:], in1=wh_sb[:],
                                                op=mybir.AluOpType.subtract)
                        nc.sync.dma_start(out=wh_d[dsl, fsl], in_=wh_sb[:])
                        nc.sync.dma_start(out=wl_d[dsl, fsl], in_=wl_sb[:])
                    else:
                        wh_sb = w_pool.tile([128, fb], BF16, tag="wp")
                        nc.sync.dma_start(out=wh_sb[:], in_=wh_d[dsl, fsl])
                        wl_sb = w_pool.tile([128, fb], BF16, tag="wp")
                        nc.sync.dma_start(out=wl_sb[:], in_=wl_d[dsl, fsl])
                    last = db == n_d - 1
                    for i in range(ns):
                        isl = slice(i * 128, (i + 1) * 128)
                        nc.tensor.matmul(pms[i][:], xhT[db][:, isl], wh_sb[:],
                                         start=(db == 0), stop=False)
                        nc.tensor.matmul(pms[i][:], xhT[db][:, isl], wl_sb[:],
                                         start=False, stop=False)
                        nc.tensor.matmul(pms[i][:], xlT[db][:, isl], wh_sb[:],
                                         start=False, stop=last)
                for i in range(ns):
                    nc.scalar.activation(acts[i][:, fsl], pms[i][:],
                                         mybir.ActivationFunctionType.Relu)

            if debug_acts:
                for i, r in enumerate(sup):
                    nc.sync.dma_start(out=acts_dbg[r * 128:(r + 1) * 128, :],
                                      in_=acts[i][:])

            # ---- per-row K-th largest via count binary search ----
            # state: lo (threshold lower bound), wdt (interval width); hi = lo + wdt
            # invariant: count(acts >= lo) >= K, count(acts >= lo + wdt) < K
            lo = sm_pool.tile([128, ns], F32, tag="sm")
            nc.vector.memset(lo[:], 0.0)
            wdt = sm_pool.tile([128, ns], F32, tag="sm")
            for i in range(ns):
                nc.vector.reduce_max(out=wdt[:, i:i + 1], in_=acts[i][:],
                                     axis=mybir.AxisListType.X)
            nc.vector.tensor_scalar(wdt[:], wdt[:], 1.0001, 1e-20,
                        op0=mybir.AluOpType.mult, op1=mybir.AluOpType.add)
            mid = sm_pool.tile([128, ns], F32, tag="sm")
            nc.vector.tensor_scalar_mul(mid[:], wdt[:], 0.5)
            cnt = sm_pool.tile([128, ns], F32, tag="sm")
            tgw = sm_pool.tile([128, ns], F32, tag="sm")
            for it in range(n_iters):
                for i in range(ns):
                    scr = scr_pool.tile([128, F], BF16, tag="scr")
                    nc.vector.tensor_scalar(scr[:], acts[i][:], mid[:, i:i + 1], None,
                                            op0=mybir.AluOpType.is_ge,
                                            op1=mybir.AluOpType.add,
                                            accum_out=cnt[:, i:i + 1])
                # wdt *= 0.5 ; lo += (cnt >= K) * wdt ; mid = 0.5*wdt + lo
                nc.vector.tensor_scalar_mul(wdt[:], wdt[:], 0.5)
                nc.vector.scalar_tensor_tensor(out=tgw[:], in0=cnt[:], scalar=float(K),
                                               in1=wdt[:], op0=mybir.AluOpType.is_ge,
                                               op1=mybir.AluOpType.mult)
                nc.vector.tensor_tensor(out=lo[:], in0=lo[:], in1=tgw[:],
                                        op=mybir.AluOpType.add)
                if it != n_iters - 1:
                    nc.vector.scalar_tensor_tensor(out=mid[:], in0=wdt[:], scalar=0.5,
                                                   in1=lo[:], op0=mybir.AluOpType.mult,
                                                   op1=mybir.AluOpType.add)
            # ---- apply mask: out = acts * (acts >= lo) ----
            for i, r in enumerate(sup):
                nc.vector.scalar_tensor_tensor(out=acts[i][:], in0=acts[i][:],
                                               scalar=lo[:, i:i + 1], in1=acts[i][:],
                                               op0=mybir.AluOpType.is_ge,
                                               op1=mybir.AluOpType.mult)
                nc.sync.dma_start(out=out[r * 128:(r + 1) * 128, :], in_=acts[i][:])
        ctx.close()

    nc.finalize()
    return nc


_NC_CACHE = {}


def _get_nc(key):
    if key not in _NC_CACHE:
        _NC_CACHE[key] = build_nc(*key)
    return _NC_CACHE[key]


def kernel(x, preencoder_bias, W_enc, b_enc):
    from concourse.bass_utils import run_bass_kernel_spmd
    x = np.asarray(x, dtype=np.float32)
    W = np.asarray(W_enc, dtype=np.float32)
    pb = np.asarray(preencoder_bias, dtype=np.float32)
    b = np.asarray(b_enc, dtype=np.float32)

    B, D = x.shape
    F = W.shape[1]
    assert (B, D, F) == (B_FULL, D_IN, N_FEAT)
    # fold biases: (x - pb) @ W + b == x @ W + (b - pb @ W)
    c = (b - pb @ W).astype(np.float32)
    if np.any(c != 0.0):
        # exact: augment the contraction with one extra 128-block where
        # x_aug[:, D] = 1 and W_aug[D, :] = c (rest zeros)
        x_aug = np.zeros((B, D + 128), dtype=np.float32)
        x_aug[:, :D] = x
        x_aug[:, D] = 1.0
        W_aug = np.zeros((D + 128, F), dtype=np.float32)
        W_aug[:D] = W
        W_aug[D] = c
        x, W, D = x_aug, W_aug, D + 128

    B_core = B // N_CORES
    nc = _get_nc((B_core, D, F, K_TOP))
    in_maps = [{"x": np.ascontiguousarray(x[i * B_core:(i + 1) * B_core]), "w": W}
               for i in range(N_CORES)]
    res = run_bass_kernel_spmd(nc, in_maps, core_ids=list(range(N_CORES)))
    return np.concatenate([res.results[i]["out"] for i in range(N_CORES)], axis=0)


# revision 13
# speedup vs baseline: 2.1972x; 1.4094x over previous
"""Trainium2 kernel: y = relu((x - pb) @ W + b) with per-row top-K threshold masking.

Strategy (per spec sharding hint): data-parallel over rows across 8 cores.
Each core computes its row shard with a 3-pass bf16 matmul decomposition
(x_hi@W_hi + x_hi@W_lo + x_lo@W_hi, f32 PSUM accumulation, ~1e-5 accurate),
then finds each row's K-th largest activation by a fused count binary search
on DVE (tensor_scalar is_ge + accumulate), which converges to 1 ulp and
reproduces jax.lax.top_k threshold masking exactly (ties included).
"""
import sys
sys.path.insert(0, "/opt/trn_rl_repo")

import numpy as np
import concourse.bass as bass
import concourse.bacc as bacc
import concourse.mybir as mybir
from concourse.tile import TileContext
from concourse.masks import make_identity

F32 = mybir.dt.float32
BF16 = mybir.dt.bfloat16
FP8 = mybir.dt.float8e4

# full problem dims (hardcoded; kernel.py must be self-contained)
B_FULL, D_IN, N_FEAT, K_TOP = 16384, 4096, 4096, 128
N_CORES = 8


def build_nc(B_core, D, F, K, n_iters=25, super_size=2, fb=512, debug_acts=False):
    assert B_core % 128 == 0 and D % 128 == 0 and F % fb == 0
    nc = bacc.Bacc("TRN2", target_bir_lowering=False, debug=True)
    x = nc.dram_tensor("x", [B_core, D], F32, kind="ExternalInput")
    w = nc.dram_tensor("w", [D, F], F32, kind="ExternalInput")
    out = nc.dram_tensor("out", [B_core, F], F32, kind="ExternalOutput")
    acts_dbg = None
    if debug_acts:
        acts_dbg = nc.dram_tensor("acts_dbg", [B_core, F], F32, kind="ExternalOutput")

    n_r = B_core // 128   # row blocks
    n_d = D // 128        # contraction blocks
    n_fb = F // fb        # feature blocks
    supers = [list(range(i, min(i + super_size, n_r)))
              for i in range(0, n_r, super_size)]

    with TileContext(nc) as tc:
        from contextlib import ExitStack
        ctx = ExitStack()
        cpool = ctx.enter_context(tc.tile_pool(name="const", bufs=1))
        dpool = ctx.enter_context(tc.tile_pool(name="wdram", bufs=1, space="DRAM"))
        xr_pool = ctx.enter_context(tc.tile_pool(name="xr", bufs=2))
        xsp_pool = ctx.enter_context(tc.tile_pool(name="xsp", bufs=2 * super_size))
        xt_pool = ctx.enter_context(tc.tile_pool(name="xt", bufs=4 * n_d))
        w_pool = ctx.enter_context(tc.tile_pool(name="wp", bufs=4))
        wsrc_pool = ctx.enter_context(tc.tile_pool(name="wsrc", bufs=2))
        acts_pool = ctx.enter_context(tc.tile_pool(name="acts", bufs=super_size + 1))
        scr_pool = ctx.enter_context(tc.tile_pool(name="scr", bufs=1))
        sm_pool = ctx.enter_context(tc.tile_pool(name="sm", bufs=4 * 6))
        mm_pool = ctx.enter_context(tc.tile_pool(name="mm", bufs=super_size + 2, space="PSUM"))
        tp_pool = ctx.enter_context(tc.tile_pool(name="tp", bufs=4, space="PSUM"))

        ident = cpool.tile([128, 128], BF16)
        make_identity(nc, ident[:])

        wh_d = dpool.tile([D, F], BF16)
        wl_d = dpool.tile([D, F], BF16)

        for si, sup in enumerate(supers):
            ns = len(sup)
            # ---- split x rows into bf16 hi/lo, then transpose via PE ----
            xh_rows, xl_rows = [], []
            for r in sup:
                xr = xr_pool.tile([128, D], F32)
                nc.sync.dma_start(out=xr[:], in_=x[r * 128:(r + 1) * 128, :])
                xh = xsp_pool.tile([128, D], BF16, tag="xsp")
                nc.vector.tensor_copy(xh[:], xr[:])
                xl = xsp_pool.tile([128, D], BF16, tag="xsp")
                nc.vector.tensor_tensor(out=xl[:], in0=xr[:], in1=xh[:],
                                        op=mybir.AluOpType.subtract)
                xh_rows.append(xh)
                xl_rows.append(xl)
            xhT, xlT = [], []
            for db in range(n_d):
                dsl = slice(db * 128, (db + 1) * 128)
                ph = tp_pool.tile([128, ns * 128], BF16, tag="tp")
                for i in range(ns):
                    nc.tensor.transpose(ph[:, i * 128:(i + 1) * 128],
                                        xh_rows[i][:, dsl], ident[:])
                th = xt_pool.tile([128, ns * 128], BF16, tag="xt")
                nc.scalar.copy(th[:], ph[:])
                xhT.append(th)
                pl = tp_pool.tile([128, ns * 128], BF16, tag="tp")
                for i in range(ns):
                    nc.tensor.transpose(pl[:, i * 128:(i + 1) * 128],
                                        xl_rows[i][:, dsl], ident[:])
                tl = xt_pool.tile([128, ns * 128], BF16, tag="xt")
                nc.scalar.copy(tl[:], pl[:])
                xlT.append(tl)

            # ---- 3-pass matmul over feature blocks ----
            acts = [acts_pool.tile([128, F], F32, tag="acts", name=f"acts{_i}") for _i in range(ns)]
            CH = 4  # d-blocks per W chunk DMA
            for f in range(n_fb):
                fsl = slice(f * fb, (f + 1) * fb)
                pms = [mm_pool.tile([128, fb], F32, tag="mm", name=f"pm{_i}") for _i in range(ns)]
                for dbc in range(n_d // CH):
                    d0 = dbc * CH * 128
                    dcsl = slice(d0, d0 + CH * 128)
                    # DRAM view [128 part, CH, fb]
                    wv = w[dcsl, fsl].rearrange("(c p) f -> p c f", p=128)
                    whv = wh_d[dcsl, fsl].rearrange("(c p) f -> p c f", p=128)
                    wlv = wl_d[dcsl, fsl].rearrange("(c p) f -> p c f", p=128)
                    if si == 0:
                        # split W on the fly (half-chunks); cache bf16 pieces in DRAM
                        wh_sb = w_pool.tile([128, CH, fb], BF16, tag="wp")
                        wl_sb = w_pool.tile([128, CH, fb], BF16, tag="wp")
                        H = CH // 2
                        for hh in range(2):
                            hsl = slice(hh * H, (hh + 1) * H)
                            wsrc = wsrc_pool.tile([128, H, fb], F32)
                            nc.sync.dma_start(out=wsrc[:], in_=wv[:, hsl, :])
                            nc.vector.tensor_copy(wh_sb[:, hsl, :], wsrc[:])
                            nc.vector.tensor_tensor(out=wl_sb[:, hsl, :], in0=wsrc[:],
                                                    in1=wh_sb[:, hsl, :],
                                                    op=mybir.AluOpType.subtract)
                        nc.sync.dma_start(out=whv, in_=wh_sb[:])
                        nc.sync.dma_start(out=wlv, in_=wl_sb[:])
                    else:
                        wh_sb = w_pool.tile([128, CH, fb], BF16, tag="wp")
                        nc.sync.dma_start(out=wh_sb[:], in_=whv)
                        wl_sb = w_pool.tile([128, CH, fb], BF16, tag="wp")
                        nc.sync.dma_start(out=wl_sb[:], in_=wlv)
                    for j in range(CH):
                        db = dbc * CH + j
                        whj = wh_sb[:, j, :]
                        wlj = wl_sb[:, j, :]
                        last = db == n_d - 1
                        for i in range(ns):
                            isl = slice(i * 128, (i + 1) * 128)
                            nc.tensor.matmul(pms[i][:], xhT[db][:, isl], whj,
                                             start=(db == 0), stop=False)
                            nc.tensor.matmul(pms[i][:], xhT[db][:, isl], wlj,
                                             start=False, stop=False)
                            nc.tensor.matmul(pms[i][:], xlT[db][:, isl], whj,
                                             start=False, stop=last)
                for i in range(ns):
                    nc.scalar.activation(acts[i][:, fsl], pms[i][:],
                                         mybir.ActivationFunctionType.Relu)

            if debug_acts:
                for i, r in enumerate(sup):
                    nc.sync.dma_start(out=acts_dbg[r * 128:(r + 1) * 128, :],
                                      in_=acts[i][:])

            # ---- per-row K-th largest via count binary search ----
            # state: lo (threshold lower bound), wdt (interval width); hi = lo + wdt
            # invariant: count(acts >= lo) >= K, count(acts >= lo + wdt) < K
            lo = sm_pool.tile([128, ns], F32, tag="sm")
            nc.vector.memset(lo[:], 0.0)
            wdt = sm_pool.tile([128, ns], F32, tag="sm")
            for i in range(ns):
                nc.vector.reduce_max(out=wdt[:, i:i + 1], in_=acts[i][:],
                                     axis=mybir.AxisListType.X)
            nc.vector.tensor_scalar(wdt[:], wdt[:], 1.0001, 1e-20,
                        op0=mybir.AluOpType.mult, op1=mybir.AluOpType.add)
            mid = sm_pool.tile([128, ns], F32, tag="sm")
            nc.vector.tensor_scalar_mul(mid[:], wdt[:], 0.5)
            cnt = sm_pool.tile([128, ns], F32, tag="sm")
            tgw = sm_pool.tile([128, ns], F32, tag="sm")
            for it in range(n_iters):
                for i in range(ns):
                    scr = scr_pool.tile([128, F], FP8, tag="scr")
                    nc.vector.tensor_scalar(scr[:], acts[i][:], mid[:, i:i + 1], None,
                                            op0=mybir.AluOpType.is_ge,
                                            op1=mybir.AluOpType.add,
                                            accum_out=cnt[:, i:i + 1])
                # wdt *= 0.5 ; lo += (cnt >= K) * wdt ; mid = 0.5*wdt + lo
                nc.vector.tensor_scalar_mul(wdt[:], wdt[:], 0.5)
                nc.vector.scalar_tensor_tensor(out=tgw[:], in0=cnt[:], scalar=float(K),
                                               in1=wdt[:], op0=mybir.AluOpType.is_ge,
                                               op1=mybir.AluOpType.mult)
                nc.vector.tensor_tensor(out=lo[:], in0=lo[:], in1=tgw[:],
                                        op=mybir.AluOpType.add)
                if it != n_iters - 1:
                    nc.vector.scalar_tensor_tensor(out=mid[:], in0=wdt[:], scalar=0.5,
                                                   in1=lo[:], op0=mybir.AluOpType.mult,
                                                   op1=mybir.AluOpType.add)
            # ---- apply mask: out = acts * (acts >= lo) ----
            for i, r in enumerate(sup):
                nc.vector.scalar_tensor_tensor(out=acts[i][:], in0=acts[i][:],
                                               scalar=lo[:, i:i + 1], in1=acts[i][:],
                                               op0=mybir.AluOpType.is_ge,
                                               op1=mybir.AluOpType.mult)
                nc.sync.dma_start(out=out[r * 128:(r + 1) * 128, :], in_=acts[i][:])
        ctx.close()

    nc.finalize()
    return nc


_NC_CACHE = {}


def _get_nc(key):
    if key not in _NC_CACHE:
        _NC_CACHE[key] = build_nc(*key)
    return _NC_CACHE[key]


def kernel(x, preencoder_bias, W_enc, b_enc):
    from concourse.bass_utils import run_bass_kernel_spmd
    x = np.asarray(x, dtype=np.float32)
    W = np.asarray(W_enc, dtype=np.float32)
    pb = np.asarray(preencoder_bias, dtype=np.float32)
    b = np.asarray(b_enc, dtype=np.float32)

    B, D = x.shape
    F = W.shape[1]
    assert (B, D, F) == (B_FULL, D_IN, N_FEAT)
    # fold biases: (x - pb) @ W + b == x @ W + (b - pb @ W)
    c = (b - pb @ W).astype(np.float32)
    if np.any(c != 0.0):
        # exact: augment the contraction with one extra 128-block where
        # x_aug[:, D] = 1 and W_aug[D, :] = c (rest zeros)
        x_aug = np.zeros((B, D + 128), dtype=np.float32)
        x_aug[:, :D] = x
        x_aug[:, D] = 1.0
        W_aug = np.zeros((D + 128, F), dtype=np.float32)
        W_aug[:D] = W
        W_aug[D] = c
        x, W, D = x_aug, W_aug, D + 128

    B_core = B // N_CORES
    nc = _get_nc((B_core, D, F, K_TOP))
    in_maps = [{"x": np.ascontiguousarray(x[i * B_core:(i + 1) * B_core]), "w": W}
               for i in range(N_CORES)]
    res = run_bass_kernel_spmd(nc, in_maps, core_ids=list(range(N_CORES)))
    return np.concatenate([res.results[i]["out"] for i in range(N_CORES)], axis=0)


# revision 17
# speedup vs baseline: 2.2903x; 1.0424x over previous
"""Trainium2 kernel: y = relu((x - pb) @ W + b) with per-row top-K threshold masking.

Strategy (per spec sharding hint): data-parallel over rows across 8 cores.
Each core computes its row shard with a 3-pass bf16 matmul decomposition
(x_hi@W_hi + x_hi@W_lo + x_lo@W_hi, f32 PSUM accumulation, ~1e-5 accurate),
then finds each row's K-th largest activation by a fused count binary search
on DVE (tensor_scalar is_ge + accumulate), which converges to 1 ulp and
reproduces jax.lax.top_k threshold masking exactly (ties included).
"""
import sys
sys.path.insert(0, "/opt/trn_rl_repo")

import numpy as np
import concourse.bass as bass
import concourse.bacc as bacc
import concourse.mybir as mybir
from concourse.tile import TileContext
from concourse.masks import make_identity

F32 = mybir.dt.float32
BF16 = mybir.dt.bfloat16
FP8 = mybir.dt.float8e4

# full problem dims (hardcoded; kernel.py must be self-contained)
B_FULL, D_IN, N_FEAT, K_TOP = 16384, 4096, 4096, 128
N_CORES = 8


def build_nc(B_core, D, F, K, n_iters=25, super_size=2, fb=512, debug_acts=False):
    assert B_core % 128 == 0 and D % 128 == 0 and F % fb == 0
    nc = bacc.Bacc("TRN2", target_bir_lowering=False, debug=True)
    x = nc.dram_tensor("x", [B_core, D], F32, kind="ExternalInput")
    w = nc.dram_tensor("w", [D, F], F32, kind="ExternalInput")
    out = nc.dram_tensor("out", [B_core, F], F32, kind="ExternalOutput")
    acts_dbg = None
    if debug_acts:
        acts_dbg = nc.dram_tensor("acts_dbg", [B_core, F], F32, kind="ExternalOutput")

    n_r = B_core // 128   # row blocks
    n_d = D // 128        # contraction blocks
    n_fb = F // fb        # feature blocks
    supers = [list(range(i, min(i + super_size, n_r)))
              for i in range(0, n_r, super_size)]

    with TileContext(nc) as tc:
        from contextlib import ExitStack
        ctx = ExitStack()
        cpool = ctx.enter_context(tc.tile_pool(name="const", bufs=1))
        dpool = ctx.enter_context(tc.tile_pool(name="wdram", bufs=1, space="DRAM"))
        xr_pool = ctx.enter_context(tc.tile_pool(name="xr", bufs=2))
        xsp_pool = ctx.enter_context(tc.tile_pool(name="xsp", bufs=2 * super_size))
        xt_pool = ctx.enter_context(tc.tile_pool(name="xt", bufs=4 * n_d))
        w_pool = ctx.enter_context(tc.tile_pool(name="wp", bufs=4))
        wsrc_pool = ctx.enter_context(tc.tile_pool(name="wsrc", bufs=2))
        acts_pool = ctx.enter_context(tc.tile_pool(name="acts", bufs=super_size + 1))
        scr_pool = ctx.enter_context(tc.tile_pool(name="scr", bufs=1))
        sm_pool = ctx.enter_context(tc.tile_pool(name="sm", bufs=4 * 6))
        mm_pool = ctx.enter_context(tc.tile_pool(name="mm", bufs=super_size + 2, space="PSUM"))
        tp_pool = ctx.enter_context(tc.tile_pool(name="tp", bufs=4, space="PSUM"))

        ident = cpool.tile([128, 128], BF16)
        make_identity(nc, ident[:])

        wh_d = dpool.tile([D, F], BF16)
        wl_d = dpool.tile([D, F], BF16)

        for si, sup in enumerate(supers):
            ns = len(sup)
            # ---- split x rows into bf16 hi/lo, then transpose via PE ----
            xh_rows, xl_rows = [], []
            for r in sup:
                xr = xr_pool.tile([128, D], F32)
                nc.sync.dma_start(out=xr[:], in_=x[r * 128:(r + 1) * 128, :])
                xh = xsp_pool.tile([128, D], BF16, tag="xsp")
                nc.vector.tensor_copy(xh[:], xr[:])
                xl = xsp_pool.tile([128, D], BF16, tag="xsp")
                nc.vector.tensor_tensor(out=xl[:], in0=xr[:], in1=xh[:],
                                        op=mybir.AluOpType.subtract)
                xh_rows.append(xh)
                xl_rows.append(xl)
            xhT, xlT = [], []
            for db in range(n_d):
                dsl = slice(db * 128, (db + 1) * 128)
                ph = tp_pool.tile([128, ns * 128], BF16, tag="tp")
                for i in range(ns):
                    nc.tensor.transpose(ph[:, i * 128:(i + 1) * 128],
                                        xh_rows[i][:, dsl], ident[:])
                th = xt_pool.tile([128, ns * 128], BF16, tag="xt")
                nc.scalar.copy(th[:], ph[:])
                xhT.append(th)
                pl = tp_pool.tile([128, ns * 128], BF16, tag="tp")
                for i in range(ns):
                    nc.tensor.transpose(pl[:, i * 128:(i + 1) * 128],
                                        xl_rows[i][:, dsl], ident[:])
                tl = xt_pool.tile([128, ns * 128], BF16, tag="xt")
                nc.scalar.copy(tl[:], pl[:])
                xlT.append(tl)

            # ---- 3-pass matmul over feature blocks ----
            acts = [acts_pool.tile([128, F], F32, tag="acts", name=f"acts{_i}") for _i in range(ns)]
            CH = 4  # d-blocks per W chunk DMA
            for f in range(n_fb):
                fsl = slice(f * fb, (f + 1) * fb)
                pms = [mm_pool.tile([128, fb], F32, tag="mm", name=f"pm{_i}") for _i in range(ns)]
                for dbc in range(n_d // CH):
                    d0 = dbc * CH * 128
                    dcsl = slice(d0, d0 + CH * 128)
                    # DRAM view [128 part, CH, fb]
                    wv = w[dcsl, fsl].rearrange("(c p) f -> p c f", p=128)
                    whv = wh_d[dcsl, fsl].rearrange("(c p) f -> p c f", p=128)
                    wlv = wl_d[dcsl, fsl].rearrange("(c p) f -> p c f", p=128)
                    if si == 0:
                        # split W on the fly (half-chunks); cache bf16 pieces in DRAM
                        wh_sb = w_pool.tile([128, CH, fb], BF16, tag="wp")
                        wl_sb = w_pool.tile([128, CH, fb], BF16, tag="wp")
                        H = CH // 2
                        for hh in range(2):
                            hsl = slice(hh * H, (hh + 1) * H)
                            wsrc = wsrc_pool.tile([128, H, fb], F32)
                            nc.sync.dma_start(out=wsrc[:], in_=wv[:, hsl, :])
                            nc.vector.tensor_copy(wh_sb[:, hsl, :], wsrc[:])
                            nc.vector.tensor_tensor(out=wl_sb[:, hsl, :], in0=wsrc[:],
                                                    in1=wh_sb[:, hsl, :],
                                                    op=mybir.AluOpType.subtract)
                        nc.sync.dma_start(out=whv, in_=wh_sb[:])
                        nc.sync.dma_start(out=wlv, in_=wl_sb[:])
                    else:
                        wh_sb = w_pool.tile([128, CH, fb], BF16, tag="wp")
                        nc.sync.dma_start(out=wh_sb[:], in_=whv)
                        wl_sb = w_pool.tile([128, CH, fb], BF16, tag="wp")
                        nc.sync.dma_start(out=wl_sb[:], in_=wlv)
                    for j in range(CH):
                        db = dbc * CH + j
                        whj = wh_sb[:, j, :]
                        wlj = wl_sb[:, j, :]
                        last = db == n_d - 1
                        for i in range(ns):
                            isl = slice(i * 128, (i + 1) * 128)
                            nc.tensor.matmul(pms[i][:], xhT[db][:, isl], whj,
                                             start=(db == 0), stop=False)
                            nc.tensor.matmul(pms[i][:], xhT[db][:, isl], wlj,
                                             start=False, stop=False)
                            nc.tensor.matmul(pms[i][:], xlT[db][:, isl], whj,
                                             start=False, stop=last)
                for i in range(ns):
                    nc.scalar.activation(acts[i][:, fsl], pms[i][:],
                                         mybir.ActivationFunctionType.Relu)

            if debug_acts:
                for i, r in enumerate(sup):
                    nc.sync.dma_start(out=acts_dbg[r * 128:(r + 1) * 128, :],
                                      in_=acts[i][:])

            # ---- per-row K-th largest via count binary search ----
            # state: lo (threshold lower bound), wdt (interval width); hi = lo + wdt
            # invariant: count(acts >= lo) >= K, count(acts >= lo + wdt) < K
            lo = sm_pool.tile([128, ns], F32, tag="sm")
            nc.vector.memset(lo[:], 0.0)
            wdt = sm_pool.tile([128, ns], F32, tag="sm")
            for i in range(ns):
                nc.vector.reduce_max(out=wdt[:, i:i + 1], in_=acts[i][:],
                                     axis=mybir.AxisListType.X)
            nc.vector.tensor_scalar(wdt[:], wdt[:], 1.0001, 1e-20,
                        op0=mybir.AluOpType.mult, op1=mybir.AluOpType.add)
            mid = sm_pool.tile([128, ns], F32, tag="sm")
            nc.vector.tensor_scalar_mul(mid[:], wdt[:], 0.5)
            cnt = sm_pool.tile([128, ns], F32, tag="sm")
            tgw = sm_pool.tile([128, ns], F32, tag="sm")
            for it in range(n_iters):
                for i in range(ns):
                    scr = scr_pool.tile([128, F], FP8, tag="scr")
                    nc.vector.tensor_scalar(scr[:], acts[i][:], mid[:, i:i + 1], None,
                                            op0=mybir.AluOpType.is_ge,
                                            op1=mybir.AluOpType.add,
                                            accum_out=cnt[:, i:i + 1])
                # wdt *= 0.5 ; lo += (cnt >= K) * wdt ; mid = 0.5*wdt + lo
                nc.vector.tensor_scalar_mul(wdt[:], wdt[:], 0.5)
                nc.vector.scalar_tensor_tensor(out=tgw[:], in0=cnt[:], scalar=float(K),
                                               in1=wdt[:], op0=mybir.AluOpType.is_ge,
                                               op1=mybir.AluOpType.mult)
                nc.vector.tensor_tensor(out=lo[:], in0=lo[:], in1=tgw[:],
                                        op=mybir.AluOpType.add)
                if it != n_iters - 1:
                    nc.vector.scalar_tensor_tensor(out=mid[:], in0=wdt[:], scalar=0.5,
                                                   in1=lo[:], op0=mybir.AluOpType.mult,
                                                   op1=mybir.AluOpType.add)
            # ---- apply mask: out = acts * (acts >= lo) ----
            for i, r in enumerate(sup):
                nc.vector.scalar_tensor_tensor(out=acts[i][:], in0=acts[i][:],
                                               scalar=lo[:, i:i + 1], in1=acts[i][:],
                                               op0=mybir.AluOpType.is_ge,
                                               op1=mybir.AluOpType.mult)
                nc.sync.dma_start(out=out[r * 128:(r + 1) * 128, :], in_=acts[i][:])
        ctx.close()

    nc.finalize()
    return nc


_NC_CACHE = {}


def _get_nc(key):
    if key not in _NC_CACHE:
        _NC_CACHE[key] = build_nc(*key)
    return _NC_CACHE[key]


def kernel(x, preencoder_bias, W_enc, b_enc):
    from concourse.bass_utils import run_bass_kernel_spmd
    x = np.asarray(x, dtype=np.float32)
    W = np.asarray(W_enc, dtype=np.float32)
    pb = np.asarray(preencoder_bias, dtype=np.float32)
    b = np.asarray(b_enc, dtype=np.float32)

    B, D = x.shape
    F = W.shape[1]
    assert (B, D, F) == (B_FULL, D_IN, N_FEAT)
    # fold biases: (x - pb) @ W + b == x @ W + (b - pb @ W)
    c = (b - pb @ W).astype(np.float32)
    if np.any(c != 0.0):
        # exact: augment the contraction with one extra 128-block where
        # x_aug[:, D] = 1 and W_aug[D, :] = c (rest zeros)
        x_aug = np.zeros((B, D + 128), dtype=np.float32)
        x_aug[:, :D] = x
        x_aug[:, D] = 1.0
        W_aug = np.zeros((D + 128, F), dtype=np.float32)
        W_aug[:D] = W
        W_aug[D] = c
        x, W, D = x_aug, W_aug, D + 128

    B_core = B // N_CORES
    nc = _get_nc((B_core, D, F, K_TOP))
    in_maps = [{"x": np.ascontiguousarray(x[i * B_core:(i + 1) * B_core]), "w": W}
               for i in range(N_CORES)]
    res = run_bass_kernel_spmd(nc, in_maps, core_ids=list(range(N_CORES)))
    return np.concatenate([res.results[i]["out"] for i in range(N_CORES)], axis=0)


# revision 18
# speedup vs baseline: 4.2657x; 1.8625x over previous
"""Trainium2 kernel: y = relu((x - pb) @ W + b) with per-row top-K threshold masking.

Strategy (per spec sharding hint): data-parallel over rows across 8 cores.
Each core computes its row shard with a 3-pass bf16 matmul decomposition
(x_hi@W_hi + x_hi@W_lo + x_lo@W_hi, f32 PSUM accumulation, ~1e-5 accurate),
then finds each row's K-th largest activation by a fused count binary search
on DVE (tensor_scalar is_ge + accumulate), which converges to 1 ulp and
reproduces jax.lax.top_k threshold masking exactly (ties included).
"""
import sys
sys.path.insert(0, "/opt/trn_rl_repo")

import numpy as np
import concourse.bass as bass
import concourse.bacc as bacc
import concourse.mybir as mybir
from concourse.tile import TileContext
from concourse.masks import make_identity

F32 = mybir.dt.float32
BF16 = mybir.dt.bfloat16
FP8 = mybir.dt.float8e4

# full problem dims (hardcoded; kernel.py must be self-contained)
B_FULL, D_IN, N_FEAT, K_TOP = 16384, 4096, 4096, 128
N_CORES = 8


def build_nc(B_core, D, F, K, n_iters=20, super_size=2, fb=512, debug_acts=False):
    assert B_core % 128 == 0 and D % 128 == 0 and F % fb == 0
    nc = bacc.Bacc("TRN2", target_bir_lowering=False, debug=True)
    x = nc.dram_tensor("x", [B_core, D], F32, kind="ExternalInput")
    w = nc.dram_tensor("w", [D, F], F32, kind="ExternalInput")
    out = nc.dram_tensor("out", [B_core, F], F32, kind="ExternalOutput")
    acts_dbg = None
    if debug_acts:
        acts_dbg = nc.dram_tensor("acts_dbg", [B_core, F], F32, kind="ExternalOutput")

    n_r = B_core // 128   # row blocks
    n_d = D // 128        # contraction blocks
    n_fb = F // fb        # feature blocks
    supers = [list(range(i, min(i + super_size, n_r)))
              for i in range(0, n_r, super_size)]

    with TileContext(nc) as tc:
        from contextlib import ExitStack
        ctx = ExitStack()
        cpool = ctx.enter_context(tc.tile_pool(name="const", bufs=1))
        dpool = ctx.enter_context(tc.tile_pool(name="wdram", bufs=1, space="DRAM"))
        xr_pool = ctx.enter_context(tc.tile_pool(name="xr", bufs=2))
        xsp_pool = ctx.enter_context(tc.tile_pool(name="xsp", bufs=2 * super_size))
        xt_pool = ctx.enter_context(tc.tile_pool(name="xt", bufs=4 * n_d))
        w_pool = ctx.enter_context(tc.tile_pool(name="wp", bufs=4))
        wsrc_pool = ctx.enter_context(tc.tile_pool(name="wsrc", bufs=2))
        acts_pool = ctx.enter_context(tc.tile_pool(name="acts", bufs=super_size + 1))
        scr_pool = ctx.enter_context(tc.tile_pool(name="scr", bufs=1))
        sm_pool = ctx.enter_context(tc.tile_pool(name="sm", bufs=4 * 6))
        mm_pool = ctx.enter_context(tc.tile_pool(name="mm", bufs=super_size + 2, space="PSUM"))
        tp_pool = ctx.enter_context(tc.tile_pool(name="tp", bufs=4, space="PSUM"))

        ident = cpool.tile([128, 128], BF16)
        make_identity(nc, ident[:])

        wh_d = dpool.tile([D, F], BF16)
        wl_d = dpool.tile([D, F], BF16)

        for si, sup in enumerate(supers):
            ns = len(sup)
            # ---- split x rows into bf16 hi/lo, then transpose via PE ----
            xh_rows, xl_rows = [], []
            for r in sup:
                xr = xr_pool.tile([128, D], F32)
                nc.sync.dma_start(out=xr[:], in_=x[r * 128:(r + 1) * 128, :])
                xh = xsp_pool.tile([128, D], BF16, tag="xsp")
                nc.vector.tensor_copy(xh[:], xr[:])
                xl = xsp_pool.tile([128, D], BF16, tag="xsp")
                nc.vector.tensor_tensor(out=xl[:], in0=xr[:], in1=xh[:],
                                        op=mybir.AluOpType.subtract)
                xh_rows.append(xh)
                xl_rows.append(xl)
            xhT, xlT = [], []
            for db in range(n_d):
                dsl = slice(db * 128, (db + 1) * 128)
                ph = tp_pool.tile([128, ns * 128], BF16, tag="tp")
                for i in range(ns):
                    nc.tensor.transpose(ph[:, i * 128:(i + 1) * 128],
                                        xh_rows[i][:, dsl], ident[:])
                th = xt_pool.tile([128, ns * 128], BF16, tag="xt")
                nc.scalar.copy(th[:], ph[:])
                xhT.append(th)
                pl = tp_pool.tile([128, ns * 128], BF16, tag="tp")
                for i in range(ns):
                    nc.tensor.transpose(pl[:, i * 128:(i + 1) * 128],
                                        xl_rows[i][:, dsl], ident[:])
                tl = xt_pool.tile([128, ns * 128], BF16, tag="xt")
                nc.scalar.copy(tl[:], pl[:])
                xlT.append(tl)

            # ---- 3-pass matmul over feature blocks ----
            acts = [acts_pool.tile([128, F], F32, tag="acts", name=f"acts{_i}") for _i in range(ns)]
            CH = 4  # d-blocks per W chunk DMA
            for f in range(n_fb):
                fsl = slice(f * fb, (f + 1) * fb)
                pms = [mm_pool.tile([128, fb], F32, tag="mm", name=f"pm{_i}") for _i in range(ns)]
                for dbc in range(n_d // CH):
                    d0 = dbc * CH * 128
                    dcsl = slice(d0, d0 + CH * 128)
                    # DRAM view [128 part, CH, fb]
                    wv = w[dcsl, fsl].rearrange("(c p) f -> p c f", p=128)
                    whv = wh_d[dcsl, fsl].rearrange("(c p) f -> p c f", p=128)
                    wlv = wl_d[dcsl, fsl].rearrange("(c p) f -> p c f", p=128)
                    if si == 0:
                        # split W on the fly (half-chunks); cache bf16 pieces in DRAM
                        wh_sb = w_pool.tile([128, CH, fb], BF16, tag="wp")
                        wl_sb = w_pool.tile([128, CH, fb], BF16, tag="wp")
                        H = CH // 2
                        for hh in range(2):
                            hsl = slice(hh * H, (hh + 1) * H)
                            wsrc = wsrc_pool.tile([128, H, fb], F32)
                            nc.sync.dma_start(out=wsrc[:], in_=wv[:, hsl, :])
                            nc.vector.tensor_copy(wh_sb[:, hsl, :], wsrc[:])
                            nc.vector.tensor_tensor(out=wl_sb[:, hsl, :], in0=wsrc[:],
                                                    in1=wh_sb[:, hsl, :],
                                                    op=mybir.AluOpType.subtract)
                        nc.sync.dma_start(out=whv, in_=wh_sb[:])
                        nc.sync.dma_start(out=wlv, in_=wl_sb[:])
                    else:
                        wh_sb = w_pool.tile([128, CH, fb], BF16, tag="wp")
                        nc.sync.dma_start(out=wh_sb[:], in_=whv)
                        wl_sb = w_pool.tile([128, CH, fb], BF16, tag="wp")
                        nc.sync.dma_start(out=wl_sb[:], in_=wlv)
                    for j in range(CH):
                        db = dbc * CH + j
                        whj = wh_sb[:, j, :]
                        wlj = wl_sb[:, j, :]
                        last = db == n_d - 1
                        for i in range(ns):
                            isl = slice(i * 128, (i + 1) * 128)
                            nc.tensor.matmul(pms[i][:], xhT[db][:, isl], whj,
                                             start=(db == 0), stop=False)
                            nc.tensor.matmul(pms[i][:], xhT[db][:, isl], wlj,
                                             start=False, stop=False)
                            nc.tensor.matmul(pms[i][:], xlT[db][:, isl], whj,
                                             start=False, stop=last)
                for i in range(ns):
                    nc.scalar.activation(acts[i][:, fsl], pms[i][:],
                                         mybir.ActivationFunctionType.Relu)

            if debug_acts:
                for i, r in enumerate(sup):
                    nc.sync.dma_start(out=acts_dbg[r * 128:(r + 1) * 128, :],
                                      in_=acts[i][:])

            # ---- per-row K-th largest via count binary search ----
            # state: lo (threshold lower bound), wdt (interval width); hi = lo + wdt
            # invariant: count(acts >= lo) >= K, count(acts >= lo + wdt) < K
            lo = sm_pool.tile([128, ns], F32, tag="sm")
            nc.vector.memset(lo[:], 0.0)
            wdt = sm_pool.tile([128, ns], F32, tag="sm")
            for i in range(ns):
                nc.vector.reduce_max(out=wdt[:, i:i + 1], in_=acts[i][:],
                                     axis=mybir.AxisListType.X)
            nc.vector.tensor_scalar(wdt[:], wdt[:], 1.0001, 1e-20,
                        op0=mybir.AluOpType.mult, op1=mybir.AluOpType.add)
            mid = sm_pool.tile([128, ns], F32, tag="sm")
            nc.vector.tensor_scalar_mul(mid[:], wdt[:], 0.5)
            cnt = sm_pool.tile([128, ns], F32, tag="sm")
            tgw = sm_pool.tile([128, ns], F32, tag="sm")
            for it in range(n_iters):
                for i in range(ns):
                    scr = scr_pool.tile([128, F], FP8, tag="scr")
                    nc.vector.tensor_scalar(scr[:], acts[i][:], mid[:, i:i + 1], None,
                                            op0=mybir.AluOpType.is_ge,
                                            op1=mybir.AluOpType.add,
                                            accum_out=cnt[:, i:i + 1])
                # wdt *= 0.5 ; lo += (cnt >= K) * wdt ; mid = 0.5*wdt + lo
                nc.vector.tensor_scalar_mul(wdt[:], wdt[:], 0.5)
                nc.vector.scalar_tensor_tensor(out=tgw[:], in0=cnt[:], scalar=float(K),
                                               in1=wdt[:], op0=mybir.AluOpType.is_ge,
                                               op1=mybir.AluOpType.mult)
                nc.vector.tensor_tensor(out=lo[:], in0=lo[:], in1=tgw[:],
                                        op=mybir.AluOpType.add)
                if it != n_iters - 1:
                    nc.vector.scalar_tensor_tensor(out=mid[:], in0=wdt[:], scalar=0.5,
                                                   in1=lo[:], op0=mybir.AluOpType.mult,
                                                   op1=mybir.AluOpType.add)
            # ---- apply mask: out = acts * (acts >= lo) ----
            for i, r in enumerate(sup):
                nc.vector.scalar_tensor_tensor(out=acts[i][:], in0=acts[i][:],
                                               scalar=lo[:, i:i + 1], in1=acts[i][:],
                                               op0=mybir.AluOpType.is_ge,
                                               op1=mybir.AluOpType.mult)
                nc.sync.dma_start(out=out[r * 128:(r + 1) * 128, :], in_=acts[i][:])
        ctx.close()

    nc.finalize()
    return nc


_NC_CACHE = {}


def _get_nc(key):
    if key not in _NC_CACHE:
        _NC_CACHE[key] = build_nc(*key)
    return _NC_CACHE[key]


def kernel(x, preencoder_bias, W_enc, b_enc):
    from concourse.bass_utils import run_bass_kernel_spmd
    x = np.asarray(x, dtype=np.float32)
    W = np.asarray(W_enc, dtype=np.float32)
    pb = np.asarray(preencoder_bias, dtype=np.float32)
    b = np.asarray(b_enc, dtype=np.float32)

    B, D = x.shape
    F = W.shape[1]
    assert (B, D, F) == (B_FULL, D_IN, N_FEAT)
    # fold biases: (x - pb) @ W + b == x @ W + (b - pb @ W)
    c = (b - pb @ W).astype(np.float32)
    if np.any(c != 0.0):
        # exact: augment the contraction with one extra 128-block where
        # x_aug[:, D] = 1 and W_aug[D, :] = c (rest zeros)
        x_aug = np.zeros((B, D + 128), dtype=np.float32)
        x_aug[:, :D] = x
        x_aug[:, D] = 1.0
        W_aug = np.zeros((D + 128, F), dtype=np.float32)
        W_aug[:D] = W
        W_aug[D] = c
        x, W, D = x_aug, W_aug, D + 128

    B_core = B // N_CORES
    nc = _get_nc((B_core, D, F, K_TOP))
    in_maps = [{"x": np.ascontiguousarray(x[i * B_core:(i + 1) * B_core]), "w": W}
               for i in range(N_CORES)]
    res = run_bass_kernel_spmd(nc, in_maps, core_ids=list(range(N_CORES)))
    return np.concatenate([res.results[i]["out"] for i in range(N_CORES)], axis=0)
